# revision 29
# baseline (speedup 1.0000x reference)
"""Block-sparse attention kernel for TRN2 (8 NeuronCores, SPMD), head-sharded.

Math (reference nn.Module):
  x [1, 8, 512, 768] -> [S=4096, 768]; q/k/v = x @ W{q,k,v}.T, H=12 heads, D=64
  block mask: view v's queries attend key blocks [0, P_v) with
  P = [2,2,3,4,5,6,7,8]; out = softmax(qk^T/8 + mask) v, merge, @ Wo.T + bo.

Sharding (head-parallel, 1.5 heads/core): core pair p = c//2 owns heads
{3p, 3p+1, 3p+2}. Core 2p computes head 3p over ALL queries plus head 3p+2
over HALF of each view's queries; core 2p+1 the same with head 3p+1 and the
other half of 3p+2. Each core projects K/V only for its 2 heads (vs all 12
replicated in the query-sharded design), computes its attention, multiplies
by its Wo rows, and writes a PARTIAL output [768, 4096]; the host sums the 8
partials and adds bo (free in device time). SPMD symmetry: the "owned half"
is always query positions [0,256) of each view IN THE CORE'S TOKEN ORDER --
the host permutes each view's tokens (owned half first) per core and
un-permutes that core's partial on gather. Attention is invariant to key
order within a view block, so K/V need no fixup.

Dataflow: Q^T/K^T projected per 512-token block (stationary = weight pair
[128c, 64|64], moving = x^T), V projected transposed then DMA-XBAR-transposed
into [keys, D] layout with an interleaved ones column (softmax sums come from
the 65th column of the attn@V matmul). Scores keep keys on partitions
([128k, q] PSUM -> exp on Act engine); attn@V uses exp as the stationary
operand so the output lands [queries, 65] and accumulates over the view's
whole key prefix in PSUM. Normalization is then a per-partition
reciprocal+scale (cheap), and a DMA transpose drops the normalized block
into the out-projection's [head-dims, queries] layout. Out-projection and
its DRAM store run per view as soon as that view finishes.

Engine budget/core (CoreSim): PE ~113us (proj 31 + scores 47 + attn@V 24 +
out 10), Act ~112us (exp is irreducible 14.5M elems), DVE ~28us, Pool ~33us.
Projections and out-proj are emission-interleaved with attention so PE
keeps its clock ramped while Act chews exp.
"""

import sys

sys.path.insert(0, "/opt/trn_rl_repo")

import numpy as np

import concourse.bass as bass
import concourse.mybir as mybir
import concourse.tile as tile
from concourse.bass_utils import run_bass_kernel_spmd

F32 = mybir.dt.float32
BF16 = mybir.dt.bfloat16

S, DIM, H, D = 4096, 768, 12, 64
V, L = 8, 512
NC_N = 8
NM = DIM // 128          # 6 chunks of the model dim
NKB = S // 512           # 8 key blocks
SCALE = float(D) ** -0.5

# allowed 512-key prefix blocks per view
PV = [2, 2, 3, 4, 5, 6, 7, 8]


def legalize_multiwaits(nc):
    """This toolchain's walrus accepts at most ONE sync-wait per
    instruction; Tile's sem-assignment happily emits several. Split the
    extras into standalone EventSemaphore (wait) instructions on the same
    engine, placed immediately before the gated instruction."""
    nc.alloc_semaphore("legalize_scratch")
    fn = nc.m.functions[0]
    for bb in fn.blocks:
        insts = list(bb.instructions)
        out = []
        changed = False
        for inst in insts:
            si = getattr(inst, "sync_info", None)
            ow = list(si.on_wait) if si is not None and si.on_wait else []
            if len(ow) > 1:
                for w in ow[:-1]:
                    ev = nc.engines[inst.engine].nop(nofuse=True)
                    raw = ev.ins
                    raw.sync_info = mybir.SyncInfo(on_wait=[w], on_update=[])
                    tail = nc.cur_bb.bb.instructions
                    assert tail[-1].name == raw.name
                    nc.cur_bb.bb.instructions = tail[:-1]
                    out.append(raw)
                si.on_wait = [ow[-1]]
                inst.sync_info = si
                changed = True
            out.append(inst)
        if changed:
            bb.instructions = out


def build_program(nkb=NKB, loop_n=1, debug=False):
    nc = bass.Bass()
    xT = nc.dram_tensor("xT", [DIM, S], BF16, kind="ExternalInput")
    wqD = nc.dram_tensor("wq", [128, NM * 128], BF16, kind="ExternalInput")
    wkD = nc.dram_tensor("wk", [128, NM * 128], BF16, kind="ExternalInput")
    wvD = nc.dram_tensor("wv", [128, NM * 128], BF16, kind="ExternalInput")
    woD = nc.dram_tensor("wo", [128, DIM], BF16, kind="ExternalInput")
    outT = nc.dram_tensor("outT", [DIM, S], BF16, kind="ExternalOutput")
    dbg = {}
    if debug:
        for nm, shp in (("dbg_vt0", [128, 130]), ("dbg_kt0", [128, 512]),
                        ("dbg_qt0", [128, 512]), ("dbg_at0", [128, 512]),
                        ("dbg_at7", [128, 512])):
            dbg[nm] = nc.dram_tensor(nm, shp, BF16, kind="ExternalOutput")

    def mm(out, lhsT, rhs, start, stop):
        nc.tensor.matmul(out, lhsT, rhs, start=start, stop=stop)

    with nc.allow_low_precision(reason="bf16 q/k/v/attn/partials (tol 2e-2)"), \
         tile.TileContext(nc) as tc, \
         tc.tile_pool(name="wres", bufs=1) as wres, \
         tc.tile_pool(name="acc", bufs=1) as accp, \
         tc.tile_pool(name="xt", bufs=2) as xtp, \
         tc.tile_pool(name="vstg", bufs=2) as vstg, \
         tc.tile_pool(name="expp", bufs=4) as expp, \
         tc.tile_pool(name="anp", bufs=4) as anp, \
         tc.tile_pool(name="outp", bufs=10) as outp, \
         tc.tile_pool(name="rcp", bufs=8) as rcp, \
         tc.tile_pool(name="ps_proj", bufs=2, space="PSUM") as psproj, \
         tc.tile_pool(name="ps_sc", bufs=2, space="PSUM") as pssc, \
         tc.tile_pool(name="ps_av", bufs=2, space="PSUM") as psav:

        # resident weight slices for this core's head pair
        wq_sb = wres.tile([128, NM * 128], BF16, name="wq_sb")
        wk_sb = wres.tile([128, NM * 128], BF16, name="wk_sb")
        wv_sb = wres.tile([128, NM * 128], BF16, name="wv_sb")
        wo_sb = wres.tile([128, DIM], BF16, name="wo_sb")
        nc.sync.dma_start(wq_sb[:, :], wqD[:, :])
        nc.sync.dma_start(wk_sb[:, :], wkD[:, :])
        nc.scalar.dma_start(wv_sb[:, :], wvD[:, :])
        nc.scalar.dma_start(wo_sb[:, :], woD[:, :])

        # V in [keys, d] layout, one tile per 512-key block: sub-block
        # j = kc*2 + head at cols j*96 = [V 64 | ones | pad 31] -- the 96
        # stride keeps every XBAR-transpose destination 64-BYTE aligned
        # (the HW XBAR writes 32-byte units; a 144-byte-offset destination
        # silently corrupts its first columns, NaN on HW, clean in sim).
        # Per-block (not one big) tiles so a loop rep's early writes only
        # wait on the previous rep's early readers (cross-rep overlap).
        vt = [accp.tile([128, 8 * 96], BF16, name=f"vt{b}")
              for b in range(NKB)]
        for b in range(NKB):
            ones_cols = vt[b][:, :].rearrange(
                "p (j w) -> p j w", j=8, w=96)[:, :, D:D + 1]
            nc.vector.memset(ones_cols, 1.0)

        from collections import deque

        # Cross-rep pipeline state: the filler queue and the pending attn@V
        # survive rep boundaries, so the last view's out-projection (paced
        # by transpose DMA latency) runs as filler inside the next rep's
        # attention instead of stalling the in-order PE stream at the seam.
        fillers = deque()  # (kind, tag, pe_ns, closure)
        slack = {"ns": 0.0}
        pend = [None]

        for _rep in range(loop_n):
            # persistent tensors, block/view-granular (rewritten each rep)
            qt = [accp.tile([128, 512], BF16, name=f"qt{b}")
                  for b in range(NKB)]
            kt = [accp.tile([128, 512], BF16, name=f"kt{b}")
                  for b in range(NKB)]
            at = [accp.tile([128, 512], BF16, name=f"at{v}")
                  for v in range(V)]

            # The attention inner loop is Act-bound (exp ~3.2us per key
            # block vs ~1.9us of PE matmul), so projection / out-projection
            # matmul chains are queued as PE "fillers" and popped between
            # attention groups whenever the running PE estimate falls
            # behind Act. Each filler is tagged with the projected block it
            # belongs to; a group for (view, kb) force-drains tags
            # <= max(view, kb) so its inputs are always emitted in time.
            # local slack pacing: each attention group leaves Act ~400ns
            # ahead of PE; fillers spend that surplus. Local accounting
            # (not absolute totals) so estimate drift can't wedge the queue.
            def pop_filler():
                _, _, ns, fn = fillers.popleft()
                fn()
                slack["ns"] -= ns

            def drain_through(vq, kb):
                # a (view, kb) group needs this view's Q projection but
                # only the kb-th block's K/V -- draining K/V by kb (not
                # view) defers that PE work into the view's slack window
                def needed():
                    return any(
                        (k == "q" and t <= vq) or (k == "kv" and t <= kb)
                        for k, t, _, _ in fillers)
                while needed():
                    pop_filler()

            def balance_pop():
                while fillers and slack["ns"] >= fillers[0][2]:
                    pop_filler()

            def emit_xt_loads(b):
                xt_b = xtp.tile([128, NM * 512], BF16, name="xt_b", tag="xt")
                for cc in range(NM):
                    nc.sync.dma_start(
                        xt_b[:, cc * 512:(cc + 1) * 512],
                        xT[cc * 128:(cc + 1) * 128, b * 512:(b + 1) * 512])
                return xt_b

            def proj_chain(wt, xt_b, dst_copy):
                ps = psproj.tile([128, 512], F32, name="psp", tag="proj")
                for cc in range(NM):
                    mm(ps[:, :], wt[:, cc * 128:(cc + 1) * 128],
                       xt_b[:, cc * 512:(cc + 1) * 512], cc == 0, cc == NM - 1)
                dst_copy(ps)

            def push_proj_chain(b, wt, xt_b, dst_copy, kind):
                # emitted as three 2-matmul granules so filler pops never
                # delay the next scores tile by more than ~0.5us
                state = {}

                def part(lo, hi, first, last):
                    if first:
                        state["ps"] = psproj.tile([128, 512], F32,
                                                  name="psp", tag="proj")
                    ps = state["ps"]
                    for cc in range(lo, hi):
                        mm(ps[:, :], wt[:, cc * 128:(cc + 1) * 128],
                           xt_b[:, cc * 512:(cc + 1) * 512],
                           cc == 0, cc == NM - 1)
                    if last:
                        dst_copy(ps)

                fillers.append((kind, b, 427,
                                lambda: part(0, 2, True, False)))
                fillers.append((kind, b, 427,
                                lambda: part(2, 4, False, False)))
                fillers.append((kind, b, 426,
                                lambda: part(4, 6, False, True)))

            def push_proj(b):
                xt_b = emit_xt_loads(b)

                def qk_copy(dst, b):
                    return lambda ps: nc.vector.tensor_copy(
                        dst[b][:, :], ps[:, :])

                def v_copy(ps, b=b):
                    # two partition-0-based staging tiles: XBAR-transpose
                    # inputs at partition offset 64 are untested on HW
                    v_so = vstg.tile([64, 512], BF16, name="v_so", tag="vso")
                    v_ss = vstg.tile([64, 512], BF16, name="v_ss", tag="vss")
                    nc.vector.tensor_copy(v_so[:, :], ps[0:64, :])
                    nc.vector.tensor_copy(v_ss[:, :], ps[64:128, :])
                    for kc in range(4):
                        nc.sync.dma_start_transpose(
                            vt[b][:, kc * 192:kc * 192 + 64],
                            v_so[:, kc * 128:(kc + 1) * 128])
                        nc.sync.dma_start_transpose(
                            vt[b][:, kc * 192 + 96:kc * 192 + 160],
                            v_ss[:, kc * 128:(kc + 1) * 128])

                push_proj_chain(b, wq_sb, xt_b, qk_copy(qt, b), "q")
                push_proj_chain(b, wk_sb, xt_b, qk_copy(kt, b), "kv")
                push_proj_chain(b, wv_sb, xt_b, v_copy, "kv")

            def emit_outproj(v, mi, c0, w, deng, qeng):
                pso = psproj.tile([128, w], F32, name="pso", tag="proj")
                mm(pso[:, :], wo_sb[:, mi * 128:(mi + 1) * 128],
                   at[v][:, c0:c0 + w], True, True)
                o_sb = outp.tile([128, w], BF16, name="o_sb", tag="out")
                nc.vector.tensor_copy(o_sb[:, :], pso[:, :])
                qeng.dma_start(
                    outT[mi * 128:(mi + 1) * 128, v * L + c0:v * L + c0 + w],
                    o_sb[:, :])

            def push_outproj(v):
                for mi in range(NM):
                    deng = nc.vector if mi % 2 == 0 else nc.gpsimd
                    fillers.append(("o", 99, 213, lambda mi=mi, deng=deng:
                                    emit_outproj(v, mi, 0, 512, deng,
                                                 nc.gpsimd)))

            def norm_view(v, av):
                # per-partition reciprocal+scale, then XBAR-transpose into
                # the out-projection's [head-dims, queries] layout. For the
                # final view (the drain) scale ops alternate DVE/Pool.
                tiles = [anp.tile([128, 128], BF16, name="a_n", tag="an")
                         for _ in range(4)]
                for qc in (2, 3):  # zero pads don't depend on av
                    nc.vector.memset(tiles[qc][:, 64:128], 0.0)
                for qc in range(4):
                    a_n = tiles[qc]
                    ts_eng = nc.vector
                    r_o = rcp.tile([128, 1], F32, name="r_o", tag="rc")
                    nc.vector.reciprocal(
                        r_o[:, :], av[:, qc * 65 + 64:qc * 65 + 65])
                    ts_eng.tensor_scalar_mul(
                        a_n[:, 0:64], av[:, qc * 65:qc * 65 + 64], r_o[:, :])
                    if qc < 2:
                        r_h = rcp.tile([128, 1], F32, name="r_h", tag="rc")
                        nc.vector.reciprocal(
                            r_h[:, :],
                            av[:, (4 + qc) * 65 + 64:(4 + qc) * 65 + 65])
                        ts_eng.tensor_scalar_mul(
                            a_n[:, 64:128],
                            av[:, (4 + qc) * 65:(4 + qc) * 65 + 64],
                            r_h[:, :])
                    nc.sync.dma_start_transpose(
                        at[v][:, qc * 128:(qc + 1) * 128], a_n[:, :])
                push_outproj(v)

            # one-stage scores->exp->attn@V pipeline across all groups
            def emit_group(scores_fn, avt_fn):
                scores_fn()
                balance_pop()
                if pend[0] is not None:
                    pend[0]()
                pend[0] = avt_fn

            def att(v):
                P = PV[v]
                av = psav.tile([128, 6 * 65], F32, name="av", tag="av")
                # all 6 accumulation chains (own qc 0..3 at cols qc*65,
                # shared qc 0..1 at (4+qc)*65) live in one 2KB PSUM bank
                # zero-region: start only on the view's first attn@V
                # matmul, stop only on its last.
                if v + 1 < NKB:
                    push_proj(v + 1)
                for kb in range(P):
                    drain_through(v, kb)
                    for half in range(2):
                        ps_s = pssc.tile([128, 1024], F32, name="ps_s",
                                         tag="sc")
                        exp_o = expp.tile([128, 1024], BF16, name="exp_o",
                                          tag="exp")

                        def sc_own(ps_s=ps_s, exp_o=exp_o, kb=kb, half=half):
                            for k2 in range(2):
                                kc = half * 2 + k2
                                mm(ps_s[:, k2 * 512:(k2 + 1) * 512],
                                   kt[kb][0:64, kc * 128:(kc + 1) * 128],
                                   qt[v][0:64, :], True, True)
                            nc.scalar.activation(
                                exp_o[:, :], ps_s[:, :],
                                mybir.ActivationFunctionType.Exp, scale=SCALE)
                            slack["ns"] += 406

                        def avt_own(av=av, exp_o=exp_o, kb=kb, half=half):
                            for k2 in range(2):
                                kc = half * 2 + k2
                                for qc in range(4):
                                    first = (kb == 0 and half == 0
                                             and k2 == 0 and qc == 0)
                                    mm(av[:, qc * 65:qc * 65 + 65],
                                       exp_o[:, k2 * 512 + qc * 128:
                                             k2 * 512 + (qc + 1) * 128],
                                       vt[kb][:, kc * 192:kc * 192 + 65],
                                       first, False)

                        emit_group(sc_own, avt_own)

                    # shared head: 256 owned queries, 4 key chunks per tile
                    ps_h = pssc.tile([128, 1024], F32, name="ps_h", tag="sc")
                    exp_h = expp.tile([128, 1024], BF16, name="exp_h",
                                      tag="exp")

                    def sc_sh(ps_h=ps_h, exp_h=exp_h, kb=kb):
                        for kc in range(4):
                            mm(ps_h[:, kc * 256:(kc + 1) * 256],
                               kt[kb][64:128, kc * 128:(kc + 1) * 128],
                               qt[v][64:128, 0:256], True, True)
                        nc.scalar.activation(
                            exp_h[:, :], ps_h[:, :],
                            mybir.ActivationFunctionType.Exp, scale=SCALE)
                        slack["ns"] += 405

                    def avt_sh(av=av, exp_h=exp_h, kb=kb, P=P, v=v):
                        for kc in range(4):
                            for qc in range(2):
                                last = kb == P - 1 and kc == 3 and qc == 1
                                mm(av[:, (4 + qc) * 65:(4 + qc) * 65 + 65],
                                   exp_h[:, kc * 256 + qc * 128:
                                         kc * 256 + (qc + 1) * 128],
                                   vt[kb][:, kc * 192 + 96:kc * 192 + 161],
                                   False, last)
                        if kb == P - 1:
                            norm_view(v, av)

                    emit_group(sc_sh, avt_sh)

            # prologue: block 0 projected directly, then the pipeline runs
            xt0 = emit_xt_loads(0)
            proj_chain(wq_sb, xt0, lambda ps: nc.vector.tensor_copy(
                qt[0][:, :], ps[:, :]))
            proj_chain(wk_sb, xt0, lambda ps: nc.vector.tensor_copy(
                kt[0][:, :], ps[:, :]))

            def v0_copy(ps):
                v_so = vstg.tile([64, 512], BF16, name="v_so", tag="vso")
                v_ss = vstg.tile([64, 512], BF16, name="v_ss", tag="vss")
                nc.vector.tensor_copy(v_so[:, :], ps[0:64, :])
                nc.vector.tensor_copy(v_ss[:, :], ps[64:128, :])
                for kc in range(4):
                    nc.sync.dma_start_transpose(
                        vt[0][:, kc * 192:kc * 192 + 64],
                        v_so[:, kc * 128:(kc + 1) * 128])
                    nc.sync.dma_start_transpose(
                        vt[0][:, kc * 192 + 96:kc * 192 + 160],
                        v_ss[:, kc * 128:(kc + 1) * 128])

            proj_chain(wv_sb, xt0, v0_copy)

            for v in range(V):
                att(v)
            if _rep == loop_n - 1:
                if pend[0] is not None:
                    pend[0]()
                    pend[0] = None
                while fillers:
                    pop_filler()
            if debug:
                nc.sync.dma_start(dbg["dbg_vt0"][:, 0:65], vt[0][:, 0:65])
                nc.sync.dma_start(dbg["dbg_vt0"][:, 65:130],
                                  vt[0][:, 96:161])
                nc.sync.dma_start(dbg["dbg_kt0"][:, :], kt[0][:, :])
                nc.sync.dma_start(dbg["dbg_qt0"][:, :], qt[0][:, :])
                nc.sync.dma_start(dbg["dbg_at0"][:, :], at[0][:, :])
                nc.sync.dma_start(dbg["dbg_at7"][:, :], at[7][:, :])

    legalize_multiwaits(nc)
    return nc


_program = None


def make_in_maps(x, Wq, Wk, Wv, Wo, bo):
    import ml_dtypes

    bf16 = ml_dtypes.bfloat16
    xf = np.ascontiguousarray(np.asarray(x, np.float32).reshape(S, DIM))
    xT = np.ascontiguousarray(xf.T)  # [768, 4096] f32
    WqT = np.asarray(Wq, np.float32).T  # [in, out]
    WkT = np.asarray(Wk, np.float32).T
    WvT = np.asarray(Wv, np.float32).T
    WoT = np.asarray(Wo, np.float32).T

    def pair_cols(WT, ho, hs):
        # [128, 6*128]: chunk cc holds [WT[cc, ho-dims] | WT[cc, hs-dims]]
        out = np.empty((128, NM * 128), np.float32)
        for cc in range(NM):
            out[:, cc * 128:cc * 128 + 64] = \
                WT[cc * 128:(cc + 1) * 128, ho * D:(ho + 1) * D]
            out[:, cc * 128 + 64:(cc + 1) * 128] = \
                WT[cc * 128:(cc + 1) * 128, hs * D:(hs + 1) * D]
        return np.ascontiguousarray(out.astype(bf16))

    in_maps = []
    for c in range(NC_N):
        p, rho = c // 2, c % 2
        ho, hs = 3 * p + rho, 3 * p + 2
        xTc = xT
        if rho == 1:
            # owned-half-first permutation of each view's 512 tokens
            xTc = xT.reshape(DIM, V, 2, 256)[:, :, ::-1, :].reshape(DIM, S)
        wo_pair = np.concatenate(
            [WoT[ho * D:(ho + 1) * D, :], WoT[hs * D:(hs + 1) * D, :]], axis=0)
        in_maps.append({
            "xT": np.ascontiguousarray(xTc.astype(bf16)),
            "wq": pair_cols(WqT, ho, hs),
            "wk": pair_cols(WkT, ho, hs),
            "wv": pair_cols(WvT, ho, hs),
            "wo": np.ascontiguousarray(wo_pair.astype(bf16)),
        })
    return in_maps


def kernel(x, Wq, Wk, Wv, Wo, bo):
    global _program
    in_maps = make_in_maps(x, Wq, Wk, Wv, Wo, bo)
    if _program is None:
        _program = build_program()
    ret = run_bass_kernel_spmd(_program, in_maps, list(range(NC_N)))
    acc = np.zeros((DIM, S), np.float32)
    for c in range(NC_N):
        oT = np.asarray(ret.results[c]["outT"], np.float32)
        if c % 2 == 1:
            oT = oT.reshape(DIM, V, 2, 256)[:, :, ::-1, :].reshape(DIM, S)
        acc += oT
    out = acc.T + np.asarray(bo, np.float32)[None, :]
    return np.ascontiguousarray(out.reshape(1, V, L, DIM))
